# revision 3
# baseline (speedup 1.0000x reference)
"""MoE layer (8 experts, top-2) on 8 TRN2 NeuronCores via expert parallelism.

Host: router (fp64 logits, top-2, gate weights), token dispatch (gather by
expert), combine (gated scatter-add). Device (one expert per core): two-layer
FFN y = gelu(x @ W1.T + b1) @ W2.T + b2 on the tokens routed to that expert,
computed in transposed layout with bf16 operands and fp32 PSUM accumulation.
"""

import sys
from contextlib import ExitStack
from functools import lru_cache

for _p in ("/opt/trn_rl_repo", "/opt/trn_rl_repo/concourse"):
    if _p not in sys.path:
        sys.path.insert(0, _p)

import ml_dtypes
import numpy as np

DIM = 1024
FF = 4096
E = 8
N_CORES = 8
PAD = 2176  # max tokens per expert (seed-0 max count is 2161); 4x512 + 1x128
GROUPS = [(0, 512), (512, 512), (1024, 512), (1536, 512), (2048, 128)]
BF16 = ml_dtypes.bfloat16


def _build_program():
    import concourse.tile as tile
    from concourse import bacc, mybir

    BF = mybir.dt.bfloat16
    F32 = mybir.dt.float32
    GELU = mybir.ActivationFunctionType.Gelu
    IDENT = mybir.ActivationFunctionType.Identity

    nc = bacc.Bacc("TRN2", target_bir_lowering=False, debug=False,
                   num_devices=N_CORES)
    xT = nc.dram_tensor("xT", [DIM, PAD], BF, kind="ExternalInput").ap()
    w1t = nc.dram_tensor("w1t", [DIM, FF], BF, kind="ExternalInput").ap()
    w2t = nc.dram_tensor("w2t", [FF, DIM], BF, kind="ExternalInput").ap()
    b1r = nc.dram_tensor("b1r", [128, FF // 128], F32, kind="ExternalInput").ap()
    b2r = nc.dram_tensor("b2r", [128, DIM // 128], F32, kind="ExternalInput").ap()
    yT = nc.dram_tensor("yT", [DIM, PAD], F32, kind="ExternalOutput").ap()

    with tile.TileContext(nc) as tc:
        with ExitStack() as ctx:
            wp = ctx.enter_context(tc.tile_pool(name="w", bufs=1))
            xp = ctx.enter_context(tc.tile_pool(name="x", bufs=2))
            hp = ctx.enter_context(tc.tile_pool(name="h", bufs=1))
            yp = ctx.enter_context(tc.tile_pool(name="y", bufs=4))
            pp = ctx.enter_context(tc.tile_pool(name="ps", bufs=4, space="PSUM"))

            w1_sb = []
            for k in range(8):
                t = wp.tile([128, FF], BF, tag=f"w1_{k}", name=f"w1sb{k}")
                nc.gpsimd.dma_start(t[:], w1t[k * 128:(k + 1) * 128, :])
                w1_sb.append(t)
            b1_sb = wp.tile([128, FF // 128], F32, tag="b1", name="b1sb")
            nc.gpsimd.dma_start(b1_sb[:], b1r[:, :])
            b2_sb = wp.tile([128, DIM // 128], F32, tag="b2", name="b2sb")
            nc.gpsimd.dma_start(b2_sb[:], b2r[:, :])

            # group-0 x before the W2 load so layer1 can start early
            g0_x = []
            for k in range(8):
                t = xp.tile([128, GROUPS[0][1]], BF, tag=f"x_{k}",
                            name=f"xsb{k}", padded_shape=[128, 512])
                nc.gpsimd.dma_start(t[:], xT[k * 128:(k + 1) * 128,
                                             0:GROUPS[0][1]])
                g0_x.append(t)

            w2_sb = []
            for k in range(32):
                t = wp.tile([128, DIM], BF, tag=f"w2_{k}", name=f"w2sb{k}")
                nc.gpsimd.dma_start(t[:], w2t[k * 128:(k + 1) * 128, :])
                w2_sb.append(t)

            for gi, (g0, tg) in enumerate(GROUPS):
                if gi == 0:
                    x_sb = g0_x
                else:
                    x_sb = []
                    for k in range(8):
                        t = xp.tile([128, tg], BF, tag=f"x_{k}",
                                    name=f"xsb{k}", padded_shape=[128, 512])
                        nc.gpsimd.dma_start(t[:], xT[k * 128:(k + 1) * 128,
                                                     g0:g0 + tg])
                        x_sb.append(t)

                h_sb = []
                for f in range(32):
                    ps = pp.tile([128, tg], F32, name="ps1",
                                 padded_shape=[128, 512])
                    for k in range(8):
                        nc.tensor.matmul(ps[:],
                                         w1_sb[k][:, f * 128:(f + 1) * 128],
                                         x_sb[k][:],
                                         start=(k == 0), stop=(k == 7))
                    h = hp.tile([128, tg], BF, tag=f"h_{f}", name=f"hsb{f}",
                                padded_shape=[128, 512])
                    nc.scalar.activation(h[:], ps[:], GELU,
                                         bias=b1_sb[:, f:f + 1])
                    h_sb.append(h)

                for d in range(8):
                    ps = pp.tile([128, tg], F32, name="ps2",
                                 padded_shape=[128, 512])
                    for k in range(32):
                        nc.tensor.matmul(ps[:],
                                         w2_sb[k][:, d * 128:(d + 1) * 128],
                                         h_sb[k][:],
                                         start=(k == 0), stop=(k == 31))
                    y = yp.tile([128, tg], F32, name="ysb",
                                padded_shape=[128, 512])
                    nc.scalar.activation(y[:], ps[:], IDENT,
                                         bias=b2_sb[:, d:d + 1])
                    nc.gpsimd.dma_start(yT[d * 128:(d + 1) * 128, g0:g0 + tg],
                                        y[:])

    nc.compile()
    return nc


@lru_cache(maxsize=1)
def _get_runner():
    """Compile the Bass program once and return (runner, nc).

    runner(in_maps) -> list of {"yT": np.ndarray} per core. Mirrors the
    multi-core branch of bass2jax.run_bass_via_pjrt but caches the jitted
    callable so repeat calls skip retrace/recompile.
    """
    import jax
    import mybir
    from jax.experimental.shard_map import shard_map
    from jax.sharding import Mesh, PartitionSpec

    from concourse import bass2jax

    nc = _build_program()
    bass2jax.install_neuronx_cc_hook()
    if nc.dbg_addr is not None:
        assert not nc.dbg_callbacks
    partition_name = nc.partition_id_tensor.name if nc.partition_id_tensor else None
    dbg_name = nc.dbg_addr.name if nc.dbg_addr is not None else None

    in_names, out_names, out_avals = [], [], []
    for alloc in nc.m.functions[0].allocations:
        if not isinstance(alloc, mybir.MemoryLocationSet):
            continue
        name = alloc.memorylocations[0].name
        if alloc.kind == "ExternalInput":
            if name != partition_name:
                in_names.append(name)
        elif alloc.kind == "ExternalOutput":
            out_names.append(name)
            out_avals.append(jax.core.ShapedArray(
                tuple(alloc.tensor_shape), mybir.dt.np(alloc.dtype)))
    n_params = len(in_names)
    n_outs = len(out_avals)
    all_names = tuple(in_names + out_names)
    if partition_name is not None:
        all_names = all_names + (partition_name,)
    donate = tuple(range(n_params, n_params + n_outs))

    def _body(*args):
        operands = list(args)
        if partition_name is not None:
            operands.append(bass2jax.partition_id_tensor())
        return tuple(bass2jax._bass_exec_p.bind(
            *operands,
            out_avals=tuple(out_avals),
            in_names=all_names,
            out_names=tuple(out_names),
            lowering_input_output_aliases=(),
            sim_require_finite=True,
            sim_require_nnan=True,
            nc=nc,
        ))

    devices = jax.devices()[:N_CORES]
    assert len(devices) == N_CORES, f"need {N_CORES} cores, got {len(devices)}"
    mesh = Mesh(np.asarray(devices), ("core",))
    specs = (PartitionSpec("core"),) * (n_params + n_outs)
    sharded = jax.jit(
        shard_map(_body, mesh=mesh, in_specs=specs,
                  out_specs=(PartitionSpec("core"),) * n_outs,
                  check_rep=False),
        donate_argnums=donate, keep_unused=True)

    def runner(in_maps):
        if dbg_name is not None:
            in_maps = [{**m, dbg_name: np.zeros((1, 2), np.uint32)}
                       for m in in_maps]
        concat_in = [
            np.concatenate([np.asarray(m[name]) for m in in_maps], axis=0)
            for name in in_names
        ]
        concat_zeros = [
            np.zeros((N_CORES * a.shape[0], *a.shape[1:]), a.dtype)
            for a in out_avals
        ]
        out_arrs = sharded(*concat_in, *concat_zeros)
        return [
            {name: np.asarray(out_arrs[i]).reshape(
                N_CORES, *out_avals[i].shape)[c]
             for i, name in enumerate(out_names)}
            for c in range(N_CORES)
        ]

    return runner, nc


def _route(xf, Wr):
    """fp64 router: returns per-expert token indices and gate weights."""
    logits = xf.astype(np.float64) @ np.asarray(Wr, dtype=np.float64).T
    order = np.argsort(-logits, axis=1, kind="stable")
    i1, i2 = order[:, 0], order[:, 1]
    n = np.arange(xf.shape[0])
    g1 = 1.0 / (1.0 + np.exp(logits[n, i2] - logits[n, i1]))
    g2 = 1.0 - g1
    toks, gates = [], []
    for e in range(E):
        idx = np.where((i1 == e) | (i2 == e))[0]
        ge = np.where(i1[idx] == e, g1[idx], g2[idx]).astype(np.float32)
        toks.append(idx)
        gates.append(ge)
    return toks, gates


def _host_ffn(xt, W1e, b1e, W2e, b2e):
    """fp32 reference-path FFN for overflow tokens (normally unused)."""
    from scipy.special import erf
    h = xt @ W1e.T + b1e
    h = (0.5 * h * (1.0 + erf(h / np.sqrt(2.0)))).astype(np.float32)
    return h @ W2e.T + b2e


def prepare_in_maps(x, Wr, W1, b1, W2, b2):
    """Host-side routing + dispatch. Returns (in_maps, toks, gates, overflow)."""
    x = np.asarray(x, dtype=np.float32)
    xf = x.reshape(-1, DIM)
    toks, gates = _route(xf, np.asarray(Wr))
    W1 = np.asarray(W1, dtype=np.float32)
    b1 = np.asarray(b1, dtype=np.float32)
    W2 = np.asarray(W2, dtype=np.float32)
    b2 = np.asarray(b2, dtype=np.float32)

    in_maps = []
    overflow = []
    for e in range(E):
        idx = toks[e]
        if len(idx) > PAD:
            overflow.append((e, idx[PAD:], gates[e][PAD:]))
            idx = idx[:PAD]
        xTe = np.zeros((DIM, PAD), dtype=BF16)
        xTe[:, :len(idx)] = xf[idx].T.astype(BF16)
        in_maps.append({
            "xT": xTe,
            "w1t": np.ascontiguousarray(W1[e].T).astype(BF16),
            "w2t": np.ascontiguousarray(W2[e].T).astype(BF16),
            "b1r": np.ascontiguousarray(b1[e].reshape(FF // 128, 128).T),
            "b2r": np.ascontiguousarray(b2[e].reshape(DIM // 128, 128).T),
        })
    return in_maps, toks, gates, overflow


def combine(outs, toks, gates, overflow, x, W1, b1, W2, b2):
    """Gated scatter-add of per-expert outputs back to token order."""
    x = np.asarray(x, dtype=np.float32)
    B, T, _ = x.shape
    xf = x.reshape(-1, DIM)
    out = np.zeros_like(xf)
    for e in range(E):
        idx = toks[e][:PAD]
        ge = gates[e][:len(idx)]
        yT = outs[e]["yT"]
        out[idx] += ge[:, None] * yT[:, :len(idx)].T
    for e, idx, ge in overflow:
        y = _host_ffn(xf[idx], np.asarray(W1[e], dtype=np.float32),
                      np.asarray(b1[e], dtype=np.float32),
                      np.asarray(W2[e], dtype=np.float32),
                      np.asarray(b2[e], dtype=np.float32))
        out[idx] += ge[:, None] * y
    return out.reshape(B, T, DIM)


def kernel(x, Wr, W1, b1, W2, b2):
    in_maps, toks, gates, overflow = prepare_in_maps(x, Wr, W1, b1, W2, b2)
    runner, _ = _get_runner()
    outs = runner(in_maps)
    return combine(outs, toks, gates, overflow, x, W1, b1, W2, b2)


# revision 6
# speedup vs baseline: 1.0379x; 1.0379x over previous
"""MoE layer (8 experts, top-2) on 8 TRN2 NeuronCores via expert parallelism.

Host: router (fp64 logits, top-2, gate weights), token dispatch (gather by
expert), combine (gated scatter-add). Device (one expert per core): two-layer
FFN y = gelu(x @ W1.T + b1) @ W2.T + b2 on the tokens routed to that expert,
computed in transposed layout with bf16 operands and fp32 PSUM accumulation.
"""

import sys
from contextlib import ExitStack
from functools import lru_cache

for _p in ("/opt/trn_rl_repo", "/opt/trn_rl_repo/concourse"):
    if _p not in sys.path:
        sys.path.insert(0, _p)

import ml_dtypes
import numpy as np

DIM = 1024
FF = 4096
E = 8
N_CORES = 8
PAD = 2176  # max tokens per expert (seed-0 max count is 2161); 4x512 + 1x128
GROUPS = [(0, 512), (512, 512), (1024, 512), (1536, 512), (2048, 128)]
BF16 = ml_dtypes.bfloat16


def _build_program():
    import concourse.tile as tile
    from concourse import bacc, mybir

    BF = mybir.dt.bfloat16
    F32 = mybir.dt.float32
    GELU = mybir.ActivationFunctionType.Gelu
    IDENT = mybir.ActivationFunctionType.Identity

    nc = bacc.Bacc("TRN2", target_bir_lowering=False, debug=False,
                   num_devices=N_CORES)
    xT = nc.dram_tensor("xT", [DIM, PAD], BF, kind="ExternalInput").ap()
    w1t = nc.dram_tensor("w1t", [DIM, FF], BF, kind="ExternalInput").ap()
    w2t = nc.dram_tensor("w2t", [FF, DIM], BF, kind="ExternalInput").ap()
    b1r = nc.dram_tensor("b1r", [128, FF // 128], F32, kind="ExternalInput").ap()
    b2r = nc.dram_tensor("b2r", [128, DIM // 128], F32, kind="ExternalInput").ap()
    yT = nc.dram_tensor("yT", [DIM, PAD], F32, kind="ExternalOutput").ap()

    with tile.TileContext(nc) as tc:
        with ExitStack() as ctx:
            wp = ctx.enter_context(tc.tile_pool(name="w", bufs=1))
            xp = ctx.enter_context(tc.tile_pool(name="x", bufs=2))
            hp = ctx.enter_context(tc.tile_pool(name="h", bufs=1))
            yp = ctx.enter_context(tc.tile_pool(name="y", bufs=4))
            pp = ctx.enter_context(tc.tile_pool(name="ps", bufs=8, space="PSUM"))

            # interleave W1 k-tiles with group-0 x tiles so the k-outer
            # layer-1 loop can start after the first pair lands
            w1_sb = []
            g0_x = []
            for k in range(8):
                t = wp.tile([128, FF], BF, tag=f"w1_{k}", name=f"w1sb{k}")
                nc.gpsimd.dma_start(t[:], w1t[k * 128:(k + 1) * 128, :])
                w1_sb.append(t)
                tx = xp.tile([128, GROUPS[0][1]], BF, tag=f"x_{k}",
                             name=f"xsb{k}", padded_shape=[128, 512])
                nc.gpsimd.dma_start(tx[:], xT[k * 128:(k + 1) * 128,
                                              0:GROUPS[0][1]])
                g0_x.append(tx)
            b1_sb = wp.tile([128, FF // 128], F32, tag="b1", name="b1sb")
            nc.gpsimd.dma_start(b1_sb[:], b1r[:, :])
            b2_sb = wp.tile([128, DIM // 128], F32, tag="b2", name="b2sb")
            nc.gpsimd.dma_start(b2_sb[:], b2r[:, :])

            w2_sb = []
            for k in range(32):
                t = wp.tile([128, DIM], BF, tag=f"w2_{k}", name=f"w2sb{k}")
                nc.gpsimd.dma_start(t[:], w2t[k * 128:(k + 1) * 128, :])
                w2_sb.append(t)

            for gi, (g0, tg) in enumerate(GROUPS):
                if gi == 0:
                    x_sb = g0_x
                else:
                    x_sb = []
                    for k in range(8):
                        t = xp.tile([128, tg], BF, tag=f"x_{k}",
                                    name=f"xsb{k}", padded_shape=[128, 512])
                        nc.gpsimd.dma_start(t[:], xT[k * 128:(k + 1) * 128,
                                                     g0:g0 + tg])
                        x_sb.append(t)

                # k-outer over chunks of 8 f-tiles: the first matmuls only
                # need w1_sb[0]/x_sb[0], so compute overlaps the weight DMA
                h_sb = []
                for fc in range(4):
                    pss = [pp.tile([128, tg], F32, name="ps1", tag="ps",
                                   padded_shape=[128, 512]) for _ in range(8)]
                    for k in range(8):
                        for j in range(8):
                            f = fc * 8 + j
                            nc.tensor.matmul(
                                pss[j][:],
                                w1_sb[k][:, f * 128:(f + 1) * 128],
                                x_sb[k][:],
                                start=(k == 0), stop=(k == 7))
                    for j in range(8):
                        f = fc * 8 + j
                        h = hp.tile([128, tg], BF, tag=f"h_{f}",
                                    name=f"hsb{f}", padded_shape=[128, 512])
                        nc.scalar.activation(h[:], pss[j][:], GELU,
                                             bias=b1_sb[:, f:f + 1])
                        h_sb.append(h)

                for d in range(8):
                    ps = pp.tile([128, tg], F32, name="ps2", tag="ps",
                                 padded_shape=[128, 512])
                    for k in range(32):
                        nc.tensor.matmul(ps[:],
                                         w2_sb[k][:, d * 128:(d + 1) * 128],
                                         h_sb[k][:],
                                         start=(k == 0), stop=(k == 31))
                    y = yp.tile([128, tg], F32, name="ysb",
                                padded_shape=[128, 512])
                    nc.scalar.activation(y[:], ps[:], IDENT,
                                         bias=b2_sb[:, d:d + 1])
                    nc.gpsimd.dma_start(yT[d * 128:(d + 1) * 128, g0:g0 + tg],
                                        y[:])

    nc.compile()
    return nc


@lru_cache(maxsize=1)
def _get_runner():
    """Compile the Bass program once and return (runner, nc).

    runner(in_maps) -> list of {"yT": np.ndarray} per core. Mirrors the
    multi-core branch of bass2jax.run_bass_via_pjrt but caches the jitted
    callable so repeat calls skip retrace/recompile.
    """
    import jax
    import mybir
    from jax.experimental.shard_map import shard_map
    from jax.sharding import Mesh, PartitionSpec

    from concourse import bass2jax

    nc = _build_program()
    bass2jax.install_neuronx_cc_hook()
    if nc.dbg_addr is not None:
        assert not nc.dbg_callbacks
    partition_name = nc.partition_id_tensor.name if nc.partition_id_tensor else None
    dbg_name = nc.dbg_addr.name if nc.dbg_addr is not None else None

    in_names, out_names, out_avals = [], [], []
    for alloc in nc.m.functions[0].allocations:
        if not isinstance(alloc, mybir.MemoryLocationSet):
            continue
        name = alloc.memorylocations[0].name
        if alloc.kind == "ExternalInput":
            if name != partition_name:
                in_names.append(name)
        elif alloc.kind == "ExternalOutput":
            out_names.append(name)
            out_avals.append(jax.core.ShapedArray(
                tuple(alloc.tensor_shape), mybir.dt.np(alloc.dtype)))
    n_params = len(in_names)
    n_outs = len(out_avals)
    all_names = tuple(in_names + out_names)
    if partition_name is not None:
        all_names = all_names + (partition_name,)
    donate = tuple(range(n_params, n_params + n_outs))

    def _body(*args):
        operands = list(args)
        if partition_name is not None:
            operands.append(bass2jax.partition_id_tensor())
        return tuple(bass2jax._bass_exec_p.bind(
            *operands,
            out_avals=tuple(out_avals),
            in_names=all_names,
            out_names=tuple(out_names),
            lowering_input_output_aliases=(),
            sim_require_finite=True,
            sim_require_nnan=True,
            nc=nc,
        ))

    devices = jax.devices()[:N_CORES]
    assert len(devices) == N_CORES, f"need {N_CORES} cores, got {len(devices)}"
    mesh = Mesh(np.asarray(devices), ("core",))
    specs = (PartitionSpec("core"),) * (n_params + n_outs)
    sharded = jax.jit(
        shard_map(_body, mesh=mesh, in_specs=specs,
                  out_specs=(PartitionSpec("core"),) * n_outs,
                  check_rep=False),
        donate_argnums=donate, keep_unused=True)

    def runner(in_maps):
        if dbg_name is not None:
            in_maps = [{**m, dbg_name: np.zeros((1, 2), np.uint32)}
                       for m in in_maps]
        concat_in = [
            np.concatenate([np.asarray(m[name]) for m in in_maps], axis=0)
            for name in in_names
        ]
        concat_zeros = [
            np.zeros((N_CORES * a.shape[0], *a.shape[1:]), a.dtype)
            for a in out_avals
        ]
        out_arrs = sharded(*concat_in, *concat_zeros)
        return [
            {name: np.asarray(out_arrs[i]).reshape(
                N_CORES, *out_avals[i].shape)[c]
             for i, name in enumerate(out_names)}
            for c in range(N_CORES)
        ]

    return runner, nc


def _route(xf, Wr):
    """fp64 router: returns per-expert token indices and gate weights."""
    logits = xf.astype(np.float64) @ np.asarray(Wr, dtype=np.float64).T
    order = np.argsort(-logits, axis=1, kind="stable")
    i1, i2 = order[:, 0], order[:, 1]
    n = np.arange(xf.shape[0])
    g1 = 1.0 / (1.0 + np.exp(logits[n, i2] - logits[n, i1]))
    g2 = 1.0 - g1
    toks, gates = [], []
    for e in range(E):
        idx = np.where((i1 == e) | (i2 == e))[0]
        ge = np.where(i1[idx] == e, g1[idx], g2[idx]).astype(np.float32)
        toks.append(idx)
        gates.append(ge)
    return toks, gates


def _host_ffn(xt, W1e, b1e, W2e, b2e):
    """fp32 reference-path FFN for overflow tokens (normally unused)."""
    from scipy.special import erf
    h = xt @ W1e.T + b1e
    h = (0.5 * h * (1.0 + erf(h / np.sqrt(2.0)))).astype(np.float32)
    return h @ W2e.T + b2e


def prepare_in_maps(x, Wr, W1, b1, W2, b2):
    """Host-side routing + dispatch. Returns (in_maps, toks, gates, overflow)."""
    x = np.asarray(x, dtype=np.float32)
    xf = x.reshape(-1, DIM)
    toks, gates = _route(xf, np.asarray(Wr))
    W1 = np.asarray(W1, dtype=np.float32)
    b1 = np.asarray(b1, dtype=np.float32)
    W2 = np.asarray(W2, dtype=np.float32)
    b2 = np.asarray(b2, dtype=np.float32)

    in_maps = []
    overflow = []
    for e in range(E):
        idx = toks[e]
        if len(idx) > PAD:
            overflow.append((e, idx[PAD:], gates[e][PAD:]))
            idx = idx[:PAD]
        xTe = np.zeros((DIM, PAD), dtype=BF16)
        xTe[:, :len(idx)] = xf[idx].T.astype(BF16)
        in_maps.append({
            "xT": xTe,
            "w1t": np.ascontiguousarray(W1[e].T).astype(BF16),
            "w2t": np.ascontiguousarray(W2[e].T).astype(BF16),
            "b1r": np.ascontiguousarray(b1[e].reshape(FF // 128, 128).T),
            "b2r": np.ascontiguousarray(b2[e].reshape(DIM // 128, 128).T),
        })
    return in_maps, toks, gates, overflow


def combine(outs, toks, gates, overflow, x, W1, b1, W2, b2):
    """Gated scatter-add of per-expert outputs back to token order."""
    x = np.asarray(x, dtype=np.float32)
    B, T, _ = x.shape
    xf = x.reshape(-1, DIM)
    out = np.zeros_like(xf)
    for e in range(E):
        idx = toks[e][:PAD]
        ge = gates[e][:len(idx)]
        yT = outs[e]["yT"]
        out[idx] += ge[:, None] * yT[:, :len(idx)].T
    for e, idx, ge in overflow:
        y = _host_ffn(xf[idx], np.asarray(W1[e], dtype=np.float32),
                      np.asarray(b1[e], dtype=np.float32),
                      np.asarray(W2[e], dtype=np.float32),
                      np.asarray(b2[e], dtype=np.float32))
        out[idx] += ge[:, None] * y
    return out.reshape(B, T, DIM)


def kernel(x, Wr, W1, b1, W2, b2):
    in_maps, toks, gates, overflow = prepare_in_maps(x, Wr, W1, b1, W2, b2)
    runner, _ = _get_runner()
    outs = runner(in_maps)
    return combine(outs, toks, gates, overflow, x, W1, b1, W2, b2)
